# revision 45
# baseline (speedup 1.0000x reference)
"""Trainium2 Bass kernel v3 for nn_CustomMoEBranch (moe_routing).

v3 redesign on top of the v2 instruction-minimized kernel (~373us/rep ->
~120us/rep measured by median-slope over 1/72/144-rep NEFFs):
 - all conv/STFT matmuls in bf16: 1 PE cycle/row instead of fp32's 4
   (float32r is 1 cyc/row in the cost model but yields garbage on real HW)
 - whole expert path in bf16 (WAF2/W1P/xcall/H/R3/out): halves SBUF
   footprint, weight-gather DMA and output DMA bytes; rel err 3.6e-3
 - software-pipelined expert loop: conv1 matmuls of pair t+1 issue before
   conv2 matmuls of pair t; H double-buffered, PSUM = P1h[128,1024]x2bufs
   + P2c[128,512]x2bufs (6 banks); PE ~95% busy in steady state
 - STFT PSUM in two 2-bank passes (cos then sin) using the 2 banks the
   expert loop leaves free, so rep r+1's gating overlaps rep r's loop
 - conv2 epilogue relu(P2+bias) on DVE via fused tensor_scalar(add,max)
   per 512-chunk (Act is saturated by the H acts; Pool cannot read PSUM)
 - gating chain latency cuts: Act table loads hoisted via dummy acts
   (only sigmoid_and_friends funcs on Act), softmax-of-2 via Sigmoid,
   OFFP/OFFu broadcast+offsets folded into accumulated PE matmuls,
   per-sample combine with per-branch Rb moves, high-priority XCt DMA
 - kernel() caches the jitted 8-core executable (repeat calls ~1.3s)
"""
import sys
if '/opt/trn_rl_repo' not in sys.path:
    sys.path.insert(0, '/opt/trn_rl_repo')
import numpy as np

import concourse.bass as bass
import concourse.mybir as mybir
import concourse.tile as tile
from concourse import bacc
from concourse.bass_utils import run_bass_kernel_spmd

FP32 = mybir.dt.float32
F32R = mybir.dt.float32r   # PE "replicated fp32": 1 cycle/row at free>=256
BF16 = mybir.dt.bfloat16
U32 = mybir.dt.uint32
AF = mybir.ActivationFunctionType
ALU = mybir.AluOpType
NPBF16 = mybir.dt.np(BF16)
# NPEXP set below EXPDT definition


def _r(ap):
    return ap.bitcast(F32R)


N_FFT = 256
HOP = 64
E = 8
L = 4096
L1 = 2048   # conv1 out length
L2 = 1024   # conv2 out length
NF = 65     # stft frames
KS = (3, 5, 7)
SPC = 8     # samples per core
N_CORES = 8
GROUP = 1   # samples per combine/output-DMA group
DVE_EPILOGUE = True
EXPDT = BF16   # expert-path dtype (BF16 or FP32)
GATDT = BF16   # gating STFT dtype (BF16 or FP32)

# WAF2 layout [2*E*64 = 1024 rows, CA2 = 1155 cols]; row j*512 + e*64 + c:
#   cols (br*3+d)*128 + j*64 + co : conv2 lhsT block-diag half
#     = wb_br[e, co, c, d]  (other half zero)
#   cols 1152+br : conv2 bias bb_br[e, c]
CA2 = 1155
# W1T2 layout [1024 rows, 192 cols]; rows 512: copy of rows 0:512.
#   row e*64 + t, col br*64 + co = conv1 im2col weight for tap-row t
#   (t = 3 - k//2 + dd for tap dd; t == 7 -> bias ba)


def host_prep_consts(inputs):
    n = np.arange(N_FFT)
    win = (0.5 - 0.5 * np.cos(2.0 * np.pi * n / N_FFT)).astype(np.float64)
    q = np.arange(129)
    ang = 2.0 * np.pi * np.outer(n, q) / N_FFT
    dc = (win[:, None] * np.cos(ang)).astype(np.float32)  # [256, 129]
    ds = (win[:, None] * np.sin(ang)).astype(np.float32)
    dsa = ds[:128, :128].copy()
    dsb = ds[128:, :128].copy()
    dsa[:, 0] = dc[:128, 128]   # nyquist cos packed into sin's q=0 col
    dsb[:, 0] = dc[128:, 128]
    npg = mybir.dt.np(GATDT)
    consts = {
        "DCa": np.ascontiguousarray(dc[:128, :128].astype(npg)),
        "DCb": np.ascontiguousarray(dc[128:, :128].astype(npg)),
        "DSa": np.ascontiguousarray(dsa.astype(npg)),
        "DSb": np.ascontiguousarray(dsb.astype(npg)),
    }
    Wg1s = (inputs["Wg1"] / NF).astype(np.float32)  # fold 1/65 mean into Wg1
    consts["Wg1a"] = np.ascontiguousarray(Wg1s[:128])          # [128, 256]
    consts["Wg1b"] = np.ascontiguousarray(Wg1s[128:129])       # [1, 256]
    consts["bg1t"] = np.ascontiguousarray(
        np.stack([inputs["bg1"][:128], inputs["bg1"][128:]], axis=1))
    consts["Wg2a"] = np.ascontiguousarray(inputs["Wg2"][:128])
    consts["Wg2b"] = np.ascontiguousarray(inputs["Wg2"][128:])
    consts["bg2c"] = np.ascontiguousarray(inputs["bg2"][:, None])
    consts["Wg3"] = np.ascontiguousarray(inputs["Wg3"])
    consts["bg3r"] = np.ascontiguousarray(inputs["bg3"][None, :])
    consts["I8"] = np.eye(8, dtype=np.float32)
    consts["I8x64"] = np.eye(8, dtype=np.float32) * 64.0
    consts["I8x8"] = np.eye(8, dtype=np.float32) * 8.0
    consts["IOTAF"] = np.arange(8, dtype=np.float32)[None, :]

    # WAF2 [1024, 1155] (bf16)
    wa = np.zeros((2, E, 64, CA2), dtype=np.float32)
    for br, k in enumerate(KS):
        wb = inputs["wb%d" % k]   # [E, 64, 64, 3] (co, ci, d)
        for d in range(3):
            blk = np.transpose(wb[:, :, :, d], (0, 2, 1))  # [E, ci, co]
            for j in range(2):
                base = (br * 3 + d) * 128 + j * 64
                wa[j, :, :, base:base + 64] = blk
        wa[:, :, :, 1152 + br] = inputs["bb%d" % k][None]
    consts["WAF2"] = np.ascontiguousarray(
        wa.reshape(2 * E * 64, CA2).astype(mybir.dt.np(EXPDT)))

    # W1P [512, 384] (bf16): row e0*64 + e1*8 + t, col br*128 + j*64 + c
    w1t = np.zeros((E, 8, 3, 64), dtype=np.float32)  # [e, t, br, c]
    for br, k in enumerate(KS):
        w1 = inputs["wa%d" % k]   # [E, 64, 1, k]
        off = 3 - k // 2
        for dd in range(k):
            w1t[:, off + dd, br, :] = w1[:, :, 0, dd]
        w1t[:, 7, br, :] = inputs["ba%d" % k]
    w1p = np.zeros((E, E, 8, 3, 2, 64), dtype=np.float32)
    w1p[:, :, :, :, 0, :] = w1t[:, None]           # j=0 <- e0
    w1p[:, :, :, :, 1, :] = w1t[None, :]           # j=1 <- e1
    consts["W1P"] = np.ascontiguousarray(
        w1p.reshape(512, 384).astype(mybir.dt.np(EXPDT)))
    return consts


def host_prep_core(x_core):
    """Per-core input tensors. x_core: [S, 4096]."""
    S = x_core.shape[0]
    # frames, padded chunk layout [128, 2048]:
    #   col = h*1024 + ch*512 + sl*65 + f   (sl = s - 4*ch)
    xr = np.pad(x_core, ((0, 0), (128, 128)), mode="reflect")
    fr = np.zeros((128, 2048), dtype=np.float32)
    f_idx = np.arange(NF) * HOP
    p_idx = np.arange(128)
    for h in range(2):
        cols = f_idx[None, :] + 128 * h + p_idx[:, None]   # [128, NF]
        g = np.transpose(xr[:, cols], (1, 0, 2))           # [128, S, NF]
        for ch in range(2):
            blk = g[:, 4 * ch:4 * ch + 4, :].reshape(128, 4 * NF)
            fr[:, h * 1024 + ch * 512:h * 1024 + ch * 512 + 4 * NF] = blk
    # pre-strided im2col [8(d), S, 2048] (bf16): xcall[d, s, o] = x_ext[s, d+2o]
    x_ext = np.zeros((S, 4104), dtype=np.float32)
    x_ext[:, 3:3 + L] = x_core
    xcall = np.empty((8, S, L1), dtype=np.float32)
    for d in range(7):
        xcall[d] = x_ext[:, d:d + 2 * L1:2]
    xcall[7] = 1.0
    return {"fr": fr.astype(mybir.dt.np(GATDT)),
            "xcall": np.ascontiguousarray(
        xcall.astype(mybir.dt.np(EXPDT)))}


def build(SPC=8, REPS=1):
    nc = bacc.Bacc("TRN2", target_bir_lowering=False, debug=False)

    d_in = {}
    for name, shape, dt in [
        ("DCa", (128, 128), GATDT), ("DCb", (128, 128), GATDT),
        ("DSa", (128, 128), GATDT), ("DSb", (128, 128), GATDT),
        ("Wg1a", (128, 256), FP32), ("Wg1b", (1, 256), FP32),
        ("bg1t", (128, 2), FP32), ("Wg2a", (128, 128), FP32),
        ("Wg2b", (128, 128), FP32), ("bg2c", (128, 1), FP32),
        ("Wg3", (128, 8), FP32), ("bg3r", (1, 8), FP32),
        ("I8", (8, 8), FP32), ("I8x64", (8, 8), FP32),
        ("I8x8", (8, 8), FP32), ("IOTAF", (1, 8), FP32),
        ("WAF2", (1024, CA2), EXPDT),
        ("W1P", (512, 384), EXPDT), ("fr", (128, 2048), GATDT),
        ("xcall", (8, SPC * L1), EXPDT),
    ]:
        d_in[name] = nc.dram_tensor(name, list(shape), dt,
                                    kind="ExternalInput")
    out_d = nc.dram_tensor("out", [SPC, 192, L2], EXPDT,
                           kind="ExternalOutput")

    with tile.TileContext(nc) as tc:
        with tc.tile_pool(name="consts", bufs=1) as cpool:
            ct = {}
            for name in ["DCa", "DCb", "DSa", "DSb", "Wg1a", "Wg1b", "bg1t",
                         "Wg2a", "Wg2b", "bg2c", "Wg3", "bg3r", "I8",
                         "I8x64", "I8x8", "IOTAF"]:
                t = cpool.tile(list(d_in[name].shape), d_in[name].dtype,
                               tag=name)
                nc.sync.dma_start(t[:], d_in[name][:])
                ct[name] = t
            ones1S = cpool.tile([1, SPC], FP32, tag="ones1S")
            nc.vector.memset(ones1S[:], 1.0)
            ct["ones1S"] = ones1S
            # IOTAMIX[p] = (p>>6)*512 + (p&63)
            iota = cpool.tile([128, 1], U32, tag="iota")
            nc.gpsimd.iota(iota[:], pattern=[[0, 1]], base=0,
                           channel_multiplier=1)
            tlo = cpool.tile([128, 1], U32, tag="tlo")
            nc.vector.tensor_scalar(tlo[:], iota[:], 63, None, ALU.bitwise_and)
            thi = cpool.tile([128, 1], U32, tag="thi")
            nc.vector.tensor_scalar(thi[:], iota[:], 6, None,
                                    ALU.logical_shift_right)
            nc.vector.tensor_scalar(thi[:], thi[:], 9, None,
                                    ALU.logical_shift_left)
            iomix = cpool.tile([128, 1], U32, tag="iomix")
            nc.vector.tensor_tensor(out=iomix[:], in0=tlo[:], in1=thi[:],
                                    op=ALU.add)
            ct["iomix"] = iomix
            # persistent H tiles: pad col 0 is only ever written here
            H0 = cpool.tile([128, 2 + L1], EXPDT, tag="H0")
            nc.vector.memset(H0[:, 0:1], 0.0)
            H1 = cpool.tile([128, 2 + L1], EXPDT, tag="H1")
            nc.vector.memset(H1[:, 0:1], 0.0)
            ct["HH"] = (H0, H1)

            with tc.tile_pool(name="gout", bufs=1) as gpool:
                for rep in range(REPS):
                    build_rep(nc, tc, d_in, out_d, ct, gpool, SPC, rep)
    nc.compile()
    return nc


def build_rep(nc, tc, d_in, out_d, ct, gpool, SPC, rep):
    W_Bs = gpool.tile([128, SPC], FP32, tag="W_Bs")
    OFFu = gpool.tile([128, SPC], U32, tag="OFFu")
    OFFP = gpool.tile([8, SPC], U32, tag="OFFP")
    XCt = gpool.tile([8, SPC * L1], EXPDT, tag="XCt")
    with tc.high_priority():
        nc.sync.dma_start(XCt[:], d_in["xcall"][:])
    HH = ct["HH"]
    R3g = tuple(gpool.tile([128, 3 * GROUP * L2], EXPDT, name="R3%d" % i,
                           tag="R3%d" % i) for i in range(2))
    Rbg = tuple(gpool.tile([64, 3 * GROUP * L2], EXPDT, name="Rb%d" % i,
                           tag="Rb%d" % i) for i in range(2))

    # ---------------- gating (batched over samples) ----------------
    with tc.tile_pool(name="gwork", bufs=1) as gw:
      with tc.tile_pool(name="gpsum", bufs=1, space="PSUM") as gp:
        FR = gw.tile([128, 2048], GATDT, tag="FR")
        nc.scalar.dma_start(FR[:], d_in["fr"][:])
        dumt = gw.tile([1, 2], FP32, tag="dumt")
        nc.scalar.activation(dumt[:, 0:1], ct["ones1S"][:, 0:1], AF.Sqrt)
        # STFT in two 2-bank PSUM chunks (cos pass, then sin pass) so the
        # next rep's gating can start while this rep's expert loop still
        # owns 6 of the 8 PSUM banks.
        AB0 = gw.tile([1, 2048], FP32, tag="AB0")
        PGc = gp.tile([128, 1024], FP32, name="PGc", tag="PGc")
        for ch in range(2):
            sl = slice(512 * ch, 512 * (ch + 1))
            nc.tensor.matmul(PGc[:, sl], ct["DCa"][:], FR[:, sl],
                             start=True, stop=False)
            nc.tensor.matmul(PGc[:, sl], ct["DCb"][:],
                             FR[:, 1024 + 512 * ch:1024 + 512 * (ch + 1)],
                             start=False, stop=True)
        T1 = gw.tile([128, 1024], FP32, tag="T1")
        nc.scalar.activation(T1[:], PGc[:], AF.Square)
        nc.scalar.activation(AB0[:, 0:1024], PGc[0:1, :], AF.Abs)
        PGs = gp.tile([128, 1024], FP32, name="PGs", tag="PGs")
        for ch in range(2):
            sl = slice(512 * ch, 512 * (ch + 1))
            nc.tensor.matmul(PGs[:, sl], ct["DSa"][:], FR[:, sl],
                             start=True, stop=False)
            nc.tensor.matmul(PGs[:, sl], ct["DSb"][:],
                             FR[:, 1024 + 512 * ch:1024 + 512 * (ch + 1)],
                             start=False, stop=True)
        T2 = gw.tile([128, 1024], FP32, tag="T2")
        nc.scalar.activation(T2[:], PGs[:], AF.Square)
        nc.vector.tensor_tensor(out=T1[:], in0=T1[:], in1=T2[:], op=ALU.add)
        MAG = gw.tile([128, 1024], FP32, tag="MAG")
        nc.scalar.activation(MAG[:], T1[:], AF.Sqrt)
        nc.scalar.activation(AB0[:, 1024:2048], PGs[0:1, :], AF.Abs)
        nc.scalar.activation(dumt[:, 1:2], ct["ones1S"][:, 0:1], AF.Sigmoid)
      with tc.tile_pool(name="gpsum1", bufs=1, space="PSUM") as gp1:
        pooled = gw.tile([128, SPC], FP32, tag="pooled")
        vm = MAG[:].rearrange("p (c q) -> p c q", c=2)[:, :, 0:4 * NF]
        vm = vm.rearrange("p c (s f) -> p c s f", s=4)
        nc.vector.tensor_reduce(
            pooled[:].rearrange("p (c s) -> p c s", c=2), vm,
            axis=mybir.AxisListType.X, op=ALU.add)
        # row-0 fix (|C_0|) and nyquist (|C_N|) in one 4D reduce
        POOL2 = gw.tile([1, 16], FP32, tag="POOL2")
        vn = AB0[:].rearrange("p (r q) -> p r q", r=2)
        vn = vn.rearrange("p r (c q) -> p r c q", c=2)[:, :, :, 0:4 * NF]
        vn = vn.rearrange("p r c (s f) -> p r c s f", s=4)
        nc.vector.tensor_reduce(
            POOL2[:].rearrange("p (r c s) -> p r c s", r=2, c=2), vn,
            axis=mybir.AxisListType.X, op=ALU.add)
        nc.vector.tensor_copy(pooled[0:1, :], POOL2[:, 0:8])
        pooledN = POOL2[:, 8:16]

        # MLP
        h1p = gp1.tile([128, 2 * SPC], FP32, tag="h1p")
        for mh in range(2):
            sl = slice(mh * SPC, (mh + 1) * SPC)
            nc.tensor.matmul(h1p[:, sl],
                             ct["Wg1a"][:, mh * 128:(mh + 1) * 128],
                             pooled[:], start=True, stop=False)
            nc.tensor.matmul(h1p[:, sl],
                             ct["Wg1b"][:, mh * 128:(mh + 1) * 128],
                             pooledN, start=False, stop=True)
        h1 = gw.tile([128, 2 * SPC], FP32, tag="h1")
        for mh in range(2):
            sl = slice(mh * SPC, (mh + 1) * SPC)
            nc.scalar.activation(h1[:, sl], h1p[:, sl], AF.Relu,
                                 bias=ct["bg1t"][:, mh:mh + 1])
        h2p = gp1.tile([128, SPC], FP32, tag="h2p")
        nc.tensor.matmul(h2p[:], ct["Wg2a"][:], h1[:, 0:SPC],
                         start=True, stop=False)
        nc.tensor.matmul(h2p[:], ct["Wg2b"][:], h1[:, SPC:2 * SPC],
                         start=False, stop=True)
        h2 = gw.tile([128, SPC], FP32, tag="h2")
        nc.scalar.activation(h2[:], h2p[:], AF.Relu, bias=ct["bg2c"][:, 0:1])
        lgp = gp1.tile([SPC, 8], FP32, tag="lgp")
        nc.tensor.matmul(lgp[:], h2[:], ct["Wg3"][:], start=True, stop=False)
        nc.tensor.matmul(lgp[:], ct["ones1S"][:], ct["bg3r"][:],
                         start=False, stop=True)
        LT = gw.tile([SPC, 8], FP32, tag="LT")
        nc.vector.tensor_copy(LT[:], lgp[:])

        # top-2 + softmax
        vals8 = gw.tile([SPC, 8], FP32, tag="vals8")
        inds8 = gw.tile([SPC, 8], U32, tag="inds8")
        nc.vector.max(vals8[:], LT[:])
        nc.vector.max_index(inds8[:], vals8[:], LT[:])
        idxf = gw.tile([SPC, 2], FP32, tag="idxf")
        nc.vector.tensor_copy(idxf[:], inds8[:, 0:2])
        dv = gw.tile([SPC, 1], FP32, tag="dv")
        nc.vector.tensor_tensor(out=dv[:], in0=vals8[:, 0:1],
                                in1=vals8[:, 1:2], op=ALU.subtract)
        onesS = gw.tile([SPC, 1], FP32, tag="onesS")
        nc.vector.memset(onesS[:], 1.0)
        wv = gw.tile([SPC, 2], FP32, tag="wv")
        nc.scalar.activation(wv[:, 0:1], dv[:], AF.Sigmoid)
        nc.vector.tensor_tensor(out=wv[:, 1:2], in0=onesS[:],
                                in1=wv[:, 0:1], op=ALU.subtract)

        # broadcast weight + expert index across partitions
        psumB = gp1.tile([128, SPC], FP32, tag="psumB")
        psumI = gp1.tile([128, SPC], FP32, tag="psumI")
        psumP = gp1.tile([8, SPC], FP32, tag="psumP")
        nc.tensor.matmul(psumP[:], idxf[:, 0:1].to_broadcast([SPC, 8]),
                         ct["I8x64"][:], start=True, stop=False)
        nc.tensor.matmul(psumP[:], idxf[:, 1:2].to_broadcast([SPC, 8]),
                         ct["I8x8"][:], start=False, stop=False)
        nc.tensor.matmul(psumP[:], ct["IOTAF"][:],
                         ct["ones1S"][:], start=False, stop=True)
        for j in range(2):
            nc.tensor.matmul(psumB[64 * j:64 * (j + 1), :],
                             wv[:, j:j + 1].to_broadcast([SPC, 64]),
                             ct["I8"][:], start=True, stop=True)
            nc.tensor.matmul(psumI[64 * j:64 * (j + 1), :],
                             idxf[:, j:j + 1].to_broadcast([SPC, 64]),
                             ct["I8"][:], start=True, stop=True)
        nc.vector.tensor_copy(W_Bs[:], psumB[:])
        nc.vector.tensor_copy(OFFu[:], psumI[:])  # fp32 -> u32 cast
        nc.vector.tensor_scalar(OFFu[:], OFFu[:], 6, None,
                                ALU.logical_shift_left)
        nc.vector.tensor_tensor(out=OFFu[:], in0=OFFu[:],
                                in1=ct["iomix"][:].to_broadcast([128, SPC]),
                                op=ALU.add)
        # OFFP = e0*64 + e1*8 + t  (all three folded into psumP matmuls)
        nc.vector.tensor_copy(OFFP[:], psumP[:])   # fp32 -> u32 cast

    # ---------------- expert main loop (software-pipelined) ----------------
    wAall = gpool.tile([128, SPC * CA2], EXPDT, tag="wAall")
    BBW = gpool.tile([128, 3 * SPC], FP32, tag="BBW")
    W1all = gpool.tile([8, SPC * 384], EXPDT, tag="W1all")
    pairs = [(s, br) for s in range(SPC) for br in range(3)]
    NP = len(pairs)

    with tc.tile_pool(name="ps1", bufs=2, space="PSUM") as ps1, \
         tc.tile_pool(name="ps2", bufs=2, space="PSUM") as ps2:
        for s in range(SPC):
            nc.gpsimd.indirect_dma_start(
                out=W1all[:, s * 384:(s + 1) * 384], out_offset=None,
                in_=d_in["W1P"][:],
                in_offset=bass.IndirectOffsetOnAxis(ap=OFFP[:, s:s + 1],
                                                    axis=0))
            nc.gpsimd.indirect_dma_start(
                out=wAall[:, s * CA2:(s + 1) * CA2], out_offset=None,
                in_=d_in["WAF2"][:],
                in_offset=bass.IndirectOffsetOnAxis(ap=OFFu[:, s:s + 1],
                                                    axis=0))

        def conv1_issue(t):
            s, br = pairs[t]
            if br == 0:
                # per-sample bias*weight, placed here so the DVE queue
                # interleaves BBW(s) with epilogues instead of heading the
                # queue with 8 gather-blocked ops
                nc.vector.tensor_scalar(
                    BBW[:, 3 * s:3 * s + 3],
                    wAall[:, s * CA2 + 1152:s * CA2 + 1155],
                    W_Bs[:, s:s + 1], None, ALU.mult)
            lhs1 = W1all[:, s * 384 + br * 128:s * 384 + (br + 1) * 128]
            hv = []
            for half in range(2):
                P1h = ps1.tile([128, 1024], FP32, name="P1h", tag="P1h")
                for c in range(2):
                    q = 2 * half + c
                    nc.tensor.matmul(
                        P1h[:, 512 * c:512 * (c + 1)], lhs1,
                        XCt[:, s * L1 + 512 * q:s * L1 + 512 * (q + 1)],
                        start=True, stop=True)
                hv.append(P1h)
            return hv

        def acts_issue(t, hv):
            s, br = pairs[t]
            Ht = HH[t % 2]
            for half in range(2):
                nc.scalar.activation(
                    Ht[:, 1 + 1024 * half:1 + 1024 * (half + 1)],
                    hv[half][:], AF.Relu, scale=W_Bs[:, s:s + 1])

        def conv2_issue(t):
            s, br = pairs[t]
            sp = s % GROUP
            R3 = R3g[(s // GROUP) % 2]
            Rb = Rbg[(s // GROUP) % 2]
            Ht = HH[t % 2]
            last = (s == SPC - 1)
            for c in range(2):
                P2c = ps2.tile([128, 512], FP32, name="P2c", tag="P2c")
                for d in range(3):
                    base = s * CA2 + (br * 3 + d) * 128
                    nc.tensor.matmul(
                        P2c[:],
                        wAall[:, base:base + 128],
                        Ht[:, d + 1024 * c:d + 1024 * c + 1024:2],
                        start=(d == 0), stop=(d == 2))
                # relu(P2 + bias) per 512-chunk on DVE
                base_col = (3 * sp + br) * L2 + 512 * c
                nc.vector.tensor_scalar(
                    out=R3[:, base_col:base_col + 512], in0=P2c[:],
                    scalar1=BBW[:, 3 * s + br:3 * s + br + 1],
                    scalar2=0.0, op0=ALU.add, op1=ALU.max)
            sl_r = slice((3 * sp + br) * L2, (3 * sp + br + 1) * L2)
            if last:
                # per-branch moves+adds only for the final sample (tail)
                nc.sync.dma_start(Rb[:, br * L2:(br + 1) * L2],
                                  R3[64:128, br * L2:(br + 1) * L2])
                nc.vector.tensor_tensor(
                    out=Rb[:, br * L2:(br + 1) * L2],
                    in0=Rb[:, br * L2:(br + 1) * L2],
                    in1=R3[0:64, br * L2:(br + 1) * L2], op=ALU.add)
            if last:
                nc.sync.dma_start(
                    out_d[s:s + 1].rearrange(
                        "g (b p) t -> p (g b) t", b=3)[:, br:br + 1, :],
                    Rb[:, br * L2:(br + 1) * L2].rearrange(
                        "p (b t) -> p b t", b=1))
            elif br == 2 and sp == GROUP - 1:
                W = 3 * GROUP * L2
                nc.sync.dma_start(Rb[:, 0:W], R3[64:128, 0:W])
                nc.vector.tensor_tensor(out=Rb[:, 0:W], in0=Rb[:, 0:W],
                                        in1=R3[0:64, 0:W], op=ALU.add)
                nc.sync.dma_start(
                    out_d[s + 1 - GROUP:s + 1].rearrange(
                        "g (b p) t -> p g b t", b=3),
                    Rb[:, 0:W].rearrange("p (g b t) -> p g b t",
                                         g=GROUP, b=3))

        hv = conv1_issue(0)
        acts_issue(0, hv)
        for t in range(NP):
            if t + 1 < NP:
                hv1 = conv1_issue(t + 1)
            conv2_issue(t)
            if t + 1 < NP:
                acts_issue(t + 1, hv1)


_cache = {}


def _get_module(SPC=8, REPS=1):
    key = (SPC, REPS)
    if key not in _cache:
        _cache[key] = build(SPC=SPC, REPS=REPS)
    return _cache[key]


def make_in_maps(inputs):
    consts = host_prep_consts(inputs)
    in_maps = []
    for c in range(N_CORES):
        m = dict(consts)
        m.update(host_prep_core(inputs["x"][SPC * c:SPC * (c + 1)]))
        in_maps.append(m)
    return in_maps


class _CachedExec:
    """Builds the jitted 8-core executable once; repeat kernel() calls only
    device_put fresh inputs and execute (mirrors bass2jax.run_bass_via_pjrt)."""

    def __init__(self, nc, n_cores):
        import jax
        from jax.sharding import Mesh, PartitionSpec, NamedSharding
        from jax.experimental.shard_map import shard_map
        from concourse import bass2jax
        from concourse.bass2jax import _bass_exec_p, partition_id_tensor

        self.jax = jax
        self.n_cores = n_cores
        bass2jax.install_neuronx_cc_hook()
        assert nc.dbg_addr is None
        partition_name = (nc.partition_id_tensor.name
                          if nc.partition_id_tensor else None)
        in_names, out_names, out_avals, self.out_shapes = [], [], [], []
        for alloc in nc.m.functions[0].allocations:
            if not isinstance(alloc, mybir.MemoryLocationSet):
                continue
            name = alloc.memorylocations[0].name
            if alloc.kind == "ExternalInput":
                if name != partition_name:
                    in_names.append(name)
            elif alloc.kind == "ExternalOutput":
                out_names.append(name)
                shape = tuple(alloc.tensor_shape)
                dtype = mybir.dt.np(alloc.dtype)
                out_avals.append(jax.core.ShapedArray(shape, dtype))
                self.out_shapes.append((shape, dtype))
        self.in_names, self.out_names = in_names, out_names
        n_params = len(in_names)
        all_in_names = list(in_names) + list(out_names)
        if partition_name is not None:
            all_in_names.append(partition_name)

        def _body(*args):
            operands = list(args)
            if partition_name is not None:
                operands.append(partition_id_tensor())
            return tuple(_bass_exec_p.bind(
                *operands, out_avals=tuple(out_avals),
                in_names=tuple(all_in_names), out_names=tuple(out_names),
                lowering_input_output_aliases=(),
                sim_require_finite=True, sim_require_nnan=True, nc=nc))

        devices = jax.devices()[:n_cores]
        self.mesh = Mesh(np.asarray(devices), ("core",))
        n_outs = len(out_avals)
        self._fn = jax.jit(
            shard_map(_body, mesh=self.mesh,
                      in_specs=(PartitionSpec("core"),) * (n_params + n_outs),
                      out_specs=(PartitionSpec("core"),) * n_outs,
                      check_rep=False),
            keep_unused=True)
        self._sh = NamedSharding(self.mesh, PartitionSpec("core"))
        self._dev_zeros = [self.jax.device_put(
            np.zeros((n_cores * s[0], *s[1:]), dt), self._sh)
            for s, dt in self.out_shapes]

    def run(self, in_maps):
        dev_in = [self.jax.device_put(
            np.concatenate([np.asarray(in_maps[c][name])
                            for c in range(self.n_cores)], axis=0), self._sh)
            for name in self.in_names]
        outs = self._fn(*dev_in, *self._dev_zeros)
        outs = [np.asarray(o) for o in outs]
        # split core-concatenated outputs back into per-core dicts
        res = []
        for c in range(self.n_cores):
            d = {}
            for (shape, _dt), name, o in zip(self.out_shapes, self.out_names,
                                             outs):
                d[name] = o[c * shape[0]:(c + 1) * shape[0]]
            res.append(d)
        return res


_exec_cache = {}


def kernel(**inputs):
    inputs = {k: np.ascontiguousarray(np.asarray(v, dtype=np.float32))
              for k, v in inputs.items()}
    nc = _get_module(SPC=SPC)
    in_maps = make_in_maps(inputs)
    try:
        if "exec" not in _exec_cache:
            _exec_cache["exec"] = _CachedExec(nc, N_CORES)
        results = _exec_cache["exec"].run(in_maps)
    except Exception:
        _exec_cache.pop("exec", None)
        res = run_bass_kernel_spmd(nc, in_maps, core_ids=list(range(N_CORES)))
        results = res.results
    return np.concatenate([r["out"].astype(np.float32) for r in results],
                          axis=0)
